# revision 12
# baseline (speedup 1.0000x reference)
"""GCN3 (3-layer graph conv + log_softmax) Trainium2 Bass kernel, 8-way SPMD.

Sharding: nodes row-sharded across 8 cores (12500 rows each); edges co-located
with their destination-row partition (host-sorted by dest tile); the [N,128]
pre-activation matrix is AllGathered to every core's DRAM each layer and edge
messages are fetched with dma_gather (512B rows, int16 indices over 4
sub-tables of 25000 rows). Segment-sum is done on the tensor engine as
S.T @ G per 128-edge chunk, where S[j,d] = vals[j] * (rows_local[j] == d)
is built in one DVE tensor_scalar op.
"""

import numpy as np

P = 128


class _Cfg:
    def __init__(self, N, NFEAT, NHID, NCLASS, CORES, EPS=1e-5,
                 GROUP_TILES=6, ROW_WINDOW=1024, NQUEUES=1, CAP_CH=6):
        self.N, self.NFEAT, self.NHID, self.NCLASS = N, NFEAT, NHID, NCLASS
        self.CORES, self.EPS = CORES, EPS
        assert N % CORES == 0
        self.NLOC = N // CORES
        self.TILES = -(-self.NLOC // P)
        self.tile_rows = [min(P, self.NLOC - t * P) for t in range(self.TILES)]
        self.KCH = -(-NFEAT // P)          # k chunks for x @ W1
        self.KPAD = self.KCH * P
        self.NSUB = -(-N // 25000) if N > 25000 else 1
        self.SUB = -(-N // self.NSUB)
        assert self.SUB <= 32768
        self.GROUP_TILES = GROUP_TILES
        self.groups = [list(range(g, min(g + GROUP_TILES, self.TILES)))
                       for g in range(0, self.TILES, GROUP_TILES)]
        self.ROW_WINDOW = ROW_WINDOW
        self.NQUEUES = NQUEUES
        self.CAP_CH = CAP_CH


class _Plan:
    """Static (data-dependent but compile-time) structure shared by all cores."""
    def __init__(self, cfg, rows, cols, vals_shape_only=None):
        # rows/cols: [3, E] int arrays (full). Chunk counts maxed over cores.
        self.cfg = cfg
        L = rows.shape[0]
        self.L = L
        self.nch = np.zeros((L, cfg.TILES, cfg.NSUB), dtype=np.int64)
        for l in range(L):
            t_of = (rows[l] % cfg.NLOC) // P
            s_of = cols[l] // cfg.SUB
            core_of = rows[l] // cfg.NLOC
            key = (core_of * cfg.TILES + t_of) * cfg.NSUB + s_of
            cnt = np.bincount(key, minlength=cfg.CORES * cfg.TILES * cfg.NSUB)
            cnt = cnt.reshape(cfg.CORES, cfg.TILES, cfg.NSUB)
            self.nch[l] = -(-cnt.max(axis=0) // P)
        # call list per layer: (g_idx, s, [(t, nch), ...], ch_off).
        # One dma_gather call must stay <= CAP_CH chunks (the SWDGE ring
        # holds ~1024 descriptors; 768-idx calls are verified safe on HW),
        # so a (group, s) segment is split into multiple calls, and a
        # (t, s) run may span calls.
        self.CAP_CH = cfg.CAP_CH
        self.calls = []
        self.tot_ch = []
        for l in range(L):
            calls_l = []
            ch_off = 0
            for gi, g in enumerate(cfg.groups):
                for s in range(cfg.NSUB):
                    pieces = []   # current call's (t, nch) list
                    acc = 0
                    for t in g:
                        left = int(self.nch[l, t, s])
                        while left > 0:
                            take = min(left, self.CAP_CH - acc)
                            pieces.append((t, take))
                            acc += take
                            left -= take
                            if acc == self.CAP_CH:
                                calls_l.append((gi, s, pieces, ch_off))
                                ch_off += acc
                                pieces, acc = [], 0
                    if acc > 0:
                        calls_l.append((gi, s, pieces, ch_off))
                        ch_off += acc
            self.calls.append(calls_l)
            self.tot_ch.append(ch_off)
        self.max_call_ch = max(
            max((sum(c for _, c in tl) for _, _, tl, _ in cl), default=1)
            for cl in self.calls)
        # group -> (chunk range, call list) for metadata loads
        self.group_spans = []
        for l in range(L):
            spans = {}
            for ci, (gi, s, tl, ch_off) in enumerate(self.calls[l]):
                n = sum(c for _, c in tl)
                lo, hi, cs = spans.get(gi, (ch_off, ch_off + n, []))
                spans[gi] = (min(lo, ch_off), max(hi, ch_off + n), cs + [ci])
            self.group_spans.append(spans)


def _preprocess(cfg, plan, x, rows, cols, vals, W1):
    """Build per-core input maps. All heavy lifting is layout/permutation."""
    L = plan.L
    # --- x shard, transposed + k-padded: [KPAD, NLOC] per core
    xT = np.zeros((cfg.CORES, cfg.KPAD, cfg.NLOC), dtype=np.float32)
    for c in range(cfg.CORES):
        xT[c, :cfg.NFEAT, :] = x[c * cfg.NLOC:(c + 1) * cfg.NLOC, :].T
    W1p = np.zeros((cfg.KPAD, cfg.NHID), dtype=np.float32)
    W1p[:cfg.NFEAT] = W1

    # --- edge data per (core, layer), ordered by call layout
    idx_arr = [[None] * L for _ in range(cfg.CORES)]
    row_arr = [[None] * L for _ in range(cfg.CORES)]
    val_arr = [[None] * L for _ in range(cfg.CORES)]
    for l in range(L):
        core_of = rows[l] // cfg.NLOC
        for c in range(cfg.CORES):
            m = core_of == c
            r = rows[l][m] - c * cfg.NLOC
            cc = cols[l][m]
            vv = vals[l][m]
            t_of = r // P
            s_of = cc // cfg.SUB
            order = np.argsort(t_of * cfg.NSUB + s_of, kind="stable")
            r, cc, vv = r[order], cc[order], vv[order]
            key = t_of[order] * cfg.NSUB + s_of[order]
            cnt = np.bincount(key, minlength=cfg.TILES * cfg.NSUB)
            off = np.zeros(cfg.TILES * cfg.NSUB + 1, dtype=np.int64)
            np.cumsum(cnt, out=off[1:])
            tot = plan.tot_ch[l] * P
            gidx = np.zeros(tot, dtype=np.int16)
            rloc = np.zeros(tot, dtype=np.float32)
            vpad = np.zeros(tot, dtype=np.float32)
            used = np.zeros(cfg.TILES * cfg.NSUB, dtype=np.int64)
            for gi, s, tl, ch_off in plan.calls[l]:
                o = ch_off * P
                for t, nch in tl:
                    k = t * cfg.NSUB + s
                    # (t,s) runs may span calls: continue from cursor
                    n = min(int(cnt[k]) - int(used[k]), nch * P)
                    if n > 0:
                        st = int(off[k]) + int(used[k])
                        sl = slice(st, st + n)
                        gidx[o:o + n] = (cc[sl] - s * cfg.SUB).astype(np.int16)
                        rloc[o:o + n] = (r[sl] - t * P).astype(np.float32)
                        vpad[o:o + n] = vv[sl]
                        used[k] += n
                    # pad slots keep gidx=0 (valid row of sub-table), val=0
                    o += nch * P
            # wrap indices: slot i -> [i%16, i//16], replicated to 128 parts
            w16 = gidx.reshape(-1, 16).T
            idx_arr[c][l] = np.tile(w16, (8, 1)).copy()
            row_arr[c][l] = rloc.reshape(-1, P).T.copy()
            val_arr[c][l] = vpad.reshape(-1, P).T.copy()
    return xT, W1p, idx_arr, row_arr, val_arr


def _build_program(cfg, plan, apply_b, apply_g, apply_lnb, apply_bout):
    import concourse.bacc as bacc
    import concourse.tile as tile
    import concourse.mybir as mybir
    from concourse import library_config

    f32 = mybir.dt.float32
    i16 = mybir.dt.int16
    Alu = mybir.AluOpType
    Act = mybir.ActivationFunctionType
    L, NH, NC = plan.L, cfg.NHID, cfg.NCLASS

    nc = bacc.Bacc("TRN2", target_bir_lowering=False, debug=False,
                   enable_asserts=False, num_devices=cfg.CORES,
                   num_swdge_queues=cfg.NQUEUES)

    # ---- I/O tensors
    xT_d = nc.dram_tensor("xT", [cfg.KPAD, cfg.NLOC], f32, kind="ExternalInput")
    W1_d = nc.dram_tensor("W1p", [cfg.KPAD, NH], f32, kind="ExternalInput")
    W2_d = nc.dram_tensor("W2", [NH, NH], f32, kind="ExternalInput")
    W3_d = nc.dram_tensor("W3", [NH, NH], f32, kind="ExternalInput")
    Wo_d = nc.dram_tensor("Wout", [NH, NC], f32, kind="ExternalInput")
    b_d = [nc.dram_tensor(f"b{l+1}", [P, NH], f32, kind="ExternalInput")
           for l in range(L)]
    bo_d = nc.dram_tensor("bout", [P, NC], f32, kind="ExternalInput")
    g_d = nc.dram_tensor("lng", [P, NH], f32, kind="ExternalInput")
    lb_d = nc.dram_tensor("lnb", [P, NH], f32, kind="ExternalInput")
    idx_d = [nc.dram_tensor(f"idx{l}", [P, plan.tot_ch[l] * 8], i16,
                            kind="ExternalInput") for l in range(L)]
    row_d = [nc.dram_tensor(f"rows{l}", [P, plan.tot_ch[l]], f32,
                            kind="ExternalInput") for l in range(L)]
    val_d = [nc.dram_tensor(f"vals{l}", [P, plan.tot_ch[l]], f32,
                            kind="ExternalInput") for l in range(L)]
    cst_d = nc.dram_tensor("consts", [P, 2 * P], f32, kind="ExternalInput")
    out_d = nc.dram_tensor("out", [cfg.NLOC, NC], f32, kind="ExternalOutput")

    W_next = [W2_d, W3_d]  # weight applied in tail of layers 0,1

    with tile.TileContext(nc) as tc:
        with tc.tile_pool(name="sb", bufs=1) as sb, \
             tc.tile_pool(name="sbG", bufs=3) as sbG, \
             tc.tile_pool(name="sbM", bufs=2) as sbM, \
             tc.tile_pool(name="sbX", bufs=2) as sbX, \
             tc.tile_pool(name="sbT", bufs=3) as sbT, \
             tc.tile_pool(name="psg", bufs=6, space="PSUM") as psg, \
             tc.tile_pool(name="pst", bufs=1, space="PSUM") as pst, \
             tc.tile_pool(name="dram", bufs=1, space="DRAM") as dram:

            nc.gpsimd.load_library(library_config.mlp)

            # ---- constants (iota row + identity, host-provided)
            cst_t = sb.tile([P, 2 * P], f32)
            nc.sync.dma_start(cst_t[:], cst_d[:])
            iota_t = cst_t[:, 0:P]
            ident = cst_t[:, P:2 * P]
            eps_t = sb.tile([P, 1], f32)
            nc.vector.memset(eps_t[:], cfg.EPS)
            W1_t = sb.tile([P, cfg.KCH * NH], f32)
            nc.sync.dma_start(
                W1_t[:].rearrange("p (k f) -> p k f", k=cfg.KCH),
                W1_d[:].rearrange("(k p) f -> p k f", p=P))
            Wn_t = []
            for l in range(L - 1):
                w = sb.tile([P, NH], f32, tag=f"wn{l}", name=f"wn{l}")
                nc.sync.dma_start(w[:], W_next[l][:])
                Wn_t.append(w)
            Wo_t = sb.tile([P, NC], f32)
            nc.sync.dma_start(Wo_t[:], Wo_d[:])
            b_t = []
            for l in range(L):
                if apply_b[l]:
                    t_ = sb.tile([P, NH], f32, tag=f"b{l}", name=f"bt{l}")
                    nc.sync.dma_start(t_[:], b_d[l][:])
                    b_t.append(t_)
                else:
                    b_t.append(None)
            g_t = lb_t = bo_t = None
            if apply_g:
                g_t = sb.tile([P, NH], f32)
                nc.sync.dma_start(g_t[:], g_d[:])
            if apply_lnb:
                lb_t = sb.tile([P, NH], f32)
                nc.sync.dma_start(lb_t[:], lb_d[:])
            if apply_bout:
                bo_t = sb.tile([P, NC], f32)
                nc.sync.dma_start(bo_t[:], bo_d[:])

            # ---- DRAM intermediates
            ag_in = [dram.tile([cfg.NLOC, NH], f32, tag=f"agin{l}",
                              name=f"agin{l}") for l in range(L)]
            aspace = "Shared" if cfg.CORES > 4 else "Local"
            pre_full = [dram.tile([cfg.N, NH], f32, tag=f"pref{l}",
                                  addr_space=aspace, name=f"pref{l}")
                        for l in range(L)]
            rg = [list(range(cfg.CORES))]

            # ---- phase 0: pre1 = x @ W1 (row-windowed)
            for w0 in range(0, cfg.NLOC, cfg.ROW_WINDOW):
                wl = min(cfg.ROW_WINDOW, cfg.NLOC - w0)
                xsl = sbX.tile([P, cfg.KCH * cfg.ROW_WINDOW], f32, tag="xsl")
                x3 = xsl[:].rearrange("p (k r) -> p k r", k=cfg.KCH)
                nc.sync.dma_start(
                    x3[:, :, :wl],
                    xT_d[:].rearrange("(k p) r -> p k r", p=P)[:, :, w0:w0 + wl])
                for t0 in range(0, wl, P):
                    t = (w0 + t0) // P
                    tl_ = cfg.tile_rows[t]
                    ps = psg.tile([P, NH], f32, space="PSUM", tag="ps")
                    for k in range(cfg.KCH):
                        nc.tensor.matmul(
                            ps[:tl_], lhsT=x3[:, k, t0:t0 + tl_],
                            rhs=W1_t[:, k * NH:(k + 1) * NH],
                            start=(k == 0), stop=(k == cfg.KCH - 1))
                    pre_sb = sbT.tile([P, NH], f32, tag="pre")
                    nc.scalar.copy(pre_sb[:tl_], ps[:tl_])
                    nc.sync.dma_start(ag_in[0][t * P:t * P + tl_, :],
                                      pre_sb[:tl_])

            # ---- layers
            for l in range(L):
                nc.gpsimd.collective_compute(
                    "AllGather", mybir.AluOpType.bypass,
                    ins=[ag_in[l].opt()], outs=[pre_full[l].opt()],
                    replica_groups=rg)

                spans = plan.group_spans[l]
                # per-tile chunk bookkeeping for start/stop flags
                tile_total = {t: int(plan.nch[l, t].sum())
                              for t in range(cfg.TILES)}
                tile_seen = {t: 0 for t in range(cfg.TILES)}
                psum_of = {}
                for gi, g in enumerate(cfg.groups):
                    lo, hi, cis = spans.get(gi, (0, 0, []))
                    gch = hi - lo
                    if gch > 0:
                        rows_t = sbM.tile([P, gch], f32, tag="rows")
                        nc.sync.dma_start(rows_t[:], row_d[l][:, lo:hi])
                        vals_t = sbM.tile([P, gch], f32, tag="vals")
                        nc.sync.dma_start(vals_t[:], val_d[l][:, lo:hi])
                        idx_t = sbM.tile([P, gch * 8], i16, tag="idx")
                        nc.sync.dma_start(idx_t[:], idx_d[l][:, lo * 8:hi * 8])

                    for ci in cis:
                        _, s, tl_list, ch_off = plan.calls[l][ci]
                        ncall = sum(c for _, c in tl_list)
                        nidx = ncall * P
                        G = sbG.tile([P, plan.CAP_CH * NH], f32, tag="G")
                        sub_rows = min(cfg.SUB, cfg.N - s * cfg.SUB)
                        nc.gpsimd.dma_gather(
                            G[:].rearrange("p (c f) -> p c f", f=NH)[:, :ncall, :],
                            pre_full[l][s * cfg.SUB:s * cfg.SUB + sub_rows, :],
                            idx_t[:, (ch_off - lo) * 8:(ch_off - lo + ncall) * 8],
                            nidx, nidx, NH,
                            queue_num=ci % cfg.NQUEUES)
                        cpos = 0
                        for t, nch in tl_list:
                            if t not in psum_of:
                                psum_of[t] = psg.tile([P, NH], f32, space="PSUM",
                                                      tag="ps", name=f"ps{t}")
                            pt = psum_of[t]
                            for j in range(nch):
                                ch = ch_off - lo + cpos + j
                                S_t = sbT.tile([P, P], f32, tag="S")
                                nc.vector.tensor_scalar(
                                    out=S_t[:], in0=iota_t,
                                    scalar1=rows_t[:, ch:ch + 1],
                                    scalar2=vals_t[:, ch:ch + 1],
                                    op0=Alu.is_equal, op1=Alu.mult)
                                nc.tensor.matmul(
                                    pt[:],
                                    lhsT=S_t[:],
                                    rhs=G[:, (cpos + j) * NH:(cpos + j + 1) * NH],
                                    start=(tile_seen[t] == 0),
                                    stop=(tile_seen[t] == tile_total[t] - 1),
                                    skip_group_check=True)
                                tile_seen[t] += 1
                            cpos += nch

                    # ---- tails for this group's tiles
                    for t in g:
                        tl_ = cfg.tile_rows[t]
                        if tile_total[t] == 0:
                            m_in = sbT.tile([P, NH], f32, tag="h")
                            nc.vector.memset(m_in[:], 0.0)
                        else:
                            m_in = psum_of.pop(t)
                        # h = m (+ b)
                        if b_t[l] is not None:
                            h_t = sbT.tile([P, NH], f32, tag="h2")
                            nc.vector.tensor_tensor(out=h_t[:], in0=m_in[:],
                                                    in1=b_t[l][:], op=Alu.add)
                        else:
                            h_t = m_in
                        stats6 = sbT.tile([P, 6], f32, tag="st6")
                        nc.vector.bn_stats(stats6[:], h_t[:])
                        stats2 = sbT.tile([P, 2], f32, tag="st2")
                        nc.vector.bn_aggr(stats2[:], stats6[:])
                        std_t = sbT.tile([P, 1], f32, tag="std")
                        nc.scalar.activation(std_t[:], stats2[:, 1:2],
                                             Act.Sqrt, bias=eps_t[:, 0:1])
                        inv_t = sbT.tile([P, 1], f32, tag="inv")
                        nc.vector.reciprocal(inv_t[:], std_t[:])
                        nms = sbT.tile([P, 1], f32, tag="nms")
                        nc.vector.tensor_scalar(
                            out=nms[:], in0=stats2[:, 0:1],
                            scalar1=inv_t[:, 0:1], scalar2=-1.0,
                            op0=Alu.mult, op1=Alu.mult)
                        t_n = sbT.tile([P, NH], f32, tag="tn")
                        nc.vector.tensor_scalar(
                            out=t_n[:], in0=h_t[:], scalar1=inv_t[:, 0:1],
                            scalar2=nms[:, 0:1], op0=Alu.mult, op1=Alu.add)
                        if g_t is not None:
                            t_g = sbT.tile([P, NH], f32, tag="tg")
                            nc.vector.tensor_tensor(out=t_g[:], in0=t_n[:],
                                                    in1=g_t[:], op=Alu.mult)
                            t_n = t_g
                        if lb_t is not None:
                            t_b = sbT.tile([P, NH], f32, tag="tb")
                            nc.vector.tensor_tensor(out=t_b[:], in0=t_n[:],
                                                    in1=lb_t[:], op=Alu.add)
                            t_n = t_b
                        hr = sbT.tile([P, NH], f32, tag="hr")
                        nc.scalar.activation(hr[:], t_n[:], Act.Relu)
                        # transpose
                        hT_ps = pst.tile([P, P], f32, space="PSUM", tag="ptr")
                        nc.tensor.transpose(out=hT_ps[:], in_=hr[:],
                                            identity=ident)
                        hT = sbT.tile([P, P], f32, tag="hT")
                        nc.scalar.copy(hT[:], hT_ps[:])
                        if l < L - 1:
                            pr_ps = pst.tile([P, NH], f32, space="PSUM",
                                             tag="ppr")
                            nc.tensor.matmul(pr_ps[:tl_], lhsT=hT[:, :tl_],
                                             rhs=Wn_t[l][:],
                                             start=True, stop=True)
                            pre_sb = sbT.tile([P, NH], f32, tag="pre")
                            nc.scalar.copy(pre_sb[:tl_], pr_ps[:tl_])
                            nc.sync.dma_start(
                                ag_in[l + 1][t * P:t * P + tl_, :],
                                pre_sb[:tl_])
                        else:
                            lg_ps = pst.tile([P, NC], f32, space="PSUM",
                                             tag="ppr")
                            nc.tensor.matmul(lg_ps[:tl_], lhsT=hT[:, :tl_],
                                             rhs=Wo_t[:], start=True,
                                             stop=True)
                            lg = sbT.tile([P, NC], f32, tag="lg")
                            if bo_t is not None:
                                nc.vector.tensor_tensor(
                                    out=lg[:tl_], in0=lg_ps[:tl_],
                                    in1=bo_t[:tl_], op=Alu.add)
                            else:
                                nc.vector.tensor_copy(lg[:tl_], lg_ps[:tl_])
                            mx = sbT.tile([P, 1], f32, tag="mx")
                            nc.vector.reduce_max(mx[:tl_], lg[:tl_],
                                                 axis=mybir.AxisListType.X)
                            nmx = sbT.tile([P, 1], f32, tag="nmx")
                            nc.vector.tensor_scalar(
                                out=nmx[:tl_], in0=mx[:tl_], scalar1=-1.0,
                                scalar2=None, op0=Alu.mult)
                            ex = sbT.tile([P, NC], f32, tag="ex")
                            se = sbT.tile([P, 1], f32, tag="se")
                            nc.scalar.activation(ex[:tl_], lg[:tl_], Act.Exp,
                                                 bias=nmx[:tl_, 0:1],
                                                 scale=1.0,
                                                 accum_out=se[:tl_, 0:1])
                            lse = sbT.tile([P, 1], f32, tag="lse")
                            nc.scalar.activation(lse[:tl_], se[:tl_], Act.Ln)
                            sh = sbT.tile([P, 1], f32, tag="sh")
                            nc.vector.tensor_tensor(out=sh[:tl_],
                                                    in0=mx[:tl_],
                                                    in1=lse[:tl_],
                                                    op=Alu.add)
                            res = sbT.tile([P, NC], f32, tag="res")
                            nc.vector.tensor_scalar(
                                out=res[:tl_], in0=lg[:tl_],
                                scalar1=sh[:tl_, 0:1], scalar2=None,
                                op0=Alu.subtract)
                            nc.sync.dma_start(
                                out_d[t * P:t * P + tl_, :], res[:tl_])
    return nc


def _prepare(x, rows, cols, vals, W1, b1, W2, b2, W3, b3,
             ln_g, ln_b, Wout, bout, cfg):
    x = np.ascontiguousarray(np.asarray(x, dtype=np.float32))
    rows = np.asarray(rows, dtype=np.int64)
    cols = np.asarray(cols, dtype=np.int64)
    vals = np.asarray(vals, dtype=np.float32)
    W1 = np.asarray(W1, dtype=np.float32)
    plan = _Plan(cfg, rows, cols)
    xT, W1p, idx_arr, row_arr, val_arr = _preprocess(
        cfg, plan, x, rows, cols, vals, W1)

    rep = np.ones((P, 1), np.float32)
    b_np = [np.asarray(b, np.float32) for b in (b1, b2, b3)]
    apply_b = [not np.all(b == 0) for b in b_np]
    ln_g = np.asarray(ln_g, np.float32)
    ln_b = np.asarray(ln_b, np.float32)
    bout = np.asarray(bout, np.float32)
    apply_g = not np.all(ln_g == 1)
    apply_lnb = not np.all(ln_b == 0)
    apply_bout = not np.all(bout == 0)

    nc = _build_program(cfg, plan, apply_b, apply_g, apply_lnb, apply_bout)
    nc.compile()

    in_maps = []
    for c in range(cfg.CORES):
        consts = np.concatenate(
            [np.tile(np.arange(P, dtype=np.float32)[None, :], (P, 1)),
             np.eye(P, dtype=np.float32)], axis=1)
        m = {
            "xT": xT[c],
            "W1p": W1p,
            "consts": np.ascontiguousarray(consts),
            "W2": np.asarray(W2, np.float32),
            "W3": np.asarray(W3, np.float32),
            "Wout": np.asarray(Wout, np.float32),
            "bout": np.ascontiguousarray(rep * bout[None, :]),
            "lng": np.ascontiguousarray(rep * ln_g[None, :]),
            "lnb": np.ascontiguousarray(rep * ln_b[None, :]),
        }
        for l in range(3):
            m[f"b{l+1}"] = np.ascontiguousarray(rep * b_np[l][None, :])
            m[f"idx{l}"] = idx_arr[c][l]
            m[f"rows{l}"] = row_arr[c][l]
            m[f"vals{l}"] = val_arr[c][l]
        in_maps.append(m)

    return nc, in_maps


def kernel(**inputs):
    from concourse.bass_utils import run_bass_kernel_spmd
    cfg = _Cfg(N=100000, NFEAT=602, NHID=128, NCLASS=41, CORES=8)
    nc, in_maps = _prepare(cfg=cfg, **inputs)
    res = run_bass_kernel_spmd(nc, in_maps, core_ids=list(range(cfg.CORES)))
    out = np.concatenate([r["out"] for r in res.results], axis=0)
    return np.ascontiguousarray(out)


# revision 13
# speedup vs baseline: 1.0128x; 1.0128x over previous
"""GCN3 (3-layer graph conv + log_softmax) Trainium2 Bass kernel, 8-way SPMD.

Sharding: nodes row-sharded across 8 cores (12500 rows each); edges co-located
with their destination-row partition (host-sorted by dest tile); the [N,128]
pre-activation matrix is AllGathered to every core's DRAM each layer and edge
messages are fetched with dma_gather (512B rows, int16 indices over 4
sub-tables of 25000 rows). Segment-sum is done on the tensor engine as
S.T @ G per 128-edge chunk, where S[j,d] = vals[j] * (rows_local[j] == d)
is built in one DVE tensor_scalar op.
"""

import numpy as np

P = 128


class _Cfg:
    def __init__(self, N, NFEAT, NHID, NCLASS, CORES, EPS=1e-5,
                 GROUP_TILES=6, ROW_WINDOW=1024, NQUEUES=2, CAP_CH=6):
        self.N, self.NFEAT, self.NHID, self.NCLASS = N, NFEAT, NHID, NCLASS
        self.CORES, self.EPS = CORES, EPS
        assert N % CORES == 0
        self.NLOC = N // CORES
        self.TILES = -(-self.NLOC // P)
        self.tile_rows = [min(P, self.NLOC - t * P) for t in range(self.TILES)]
        self.KCH = -(-NFEAT // P)          # k chunks for x @ W1
        self.KPAD = self.KCH * P
        self.NSUB = -(-N // 25000) if N > 25000 else 1
        self.SUB = -(-N // self.NSUB)
        assert self.SUB <= 32768
        self.GROUP_TILES = GROUP_TILES
        self.groups = [list(range(g, min(g + GROUP_TILES, self.TILES)))
                       for g in range(0, self.TILES, GROUP_TILES)]
        self.ROW_WINDOW = ROW_WINDOW
        self.NQUEUES = NQUEUES
        self.CAP_CH = CAP_CH


class _Plan:
    """Static (data-dependent but compile-time) structure shared by all cores."""
    def __init__(self, cfg, rows, cols, vals_shape_only=None):
        # rows/cols: [3, E] int arrays (full). Chunk counts maxed over cores.
        self.cfg = cfg
        L = rows.shape[0]
        self.L = L
        self.nch = np.zeros((L, cfg.TILES, cfg.NSUB), dtype=np.int64)
        for l in range(L):
            t_of = (rows[l] % cfg.NLOC) // P
            s_of = cols[l] // cfg.SUB
            core_of = rows[l] // cfg.NLOC
            key = (core_of * cfg.TILES + t_of) * cfg.NSUB + s_of
            cnt = np.bincount(key, minlength=cfg.CORES * cfg.TILES * cfg.NSUB)
            cnt = cnt.reshape(cfg.CORES, cfg.TILES, cfg.NSUB)
            self.nch[l] = -(-cnt.max(axis=0) // P)
        # call list per layer: (g_idx, s, [(t, nch), ...], ch_off).
        # One dma_gather call must stay <= CAP_CH chunks (the SWDGE ring
        # holds ~1024 descriptors; 768-idx calls are verified safe on HW),
        # so a (group, s) segment is split into multiple calls, and a
        # (t, s) run may span calls.
        self.CAP_CH = cfg.CAP_CH
        self.calls = []
        self.tot_ch = []
        for l in range(L):
            calls_l = []
            ch_off = 0
            for gi, g in enumerate(cfg.groups):
                for s in range(cfg.NSUB):
                    pieces = []   # current call's (t, nch) list
                    acc = 0
                    for t in g:
                        left = int(self.nch[l, t, s])
                        while left > 0:
                            take = min(left, self.CAP_CH - acc)
                            pieces.append((t, take))
                            acc += take
                            left -= take
                            if acc == self.CAP_CH:
                                calls_l.append((gi, s, pieces, ch_off))
                                ch_off += acc
                                pieces, acc = [], 0
                    if acc > 0:
                        calls_l.append((gi, s, pieces, ch_off))
                        ch_off += acc
            self.calls.append(calls_l)
            self.tot_ch.append(ch_off)
        self.max_call_ch = max(
            max((sum(c for _, c in tl) for _, _, tl, _ in cl), default=1)
            for cl in self.calls)
        # group -> (chunk range, call list) for metadata loads
        self.group_spans = []
        for l in range(L):
            spans = {}
            for ci, (gi, s, tl, ch_off) in enumerate(self.calls[l]):
                n = sum(c for _, c in tl)
                lo, hi, cs = spans.get(gi, (ch_off, ch_off + n, []))
                spans[gi] = (min(lo, ch_off), max(hi, ch_off + n), cs + [ci])
            self.group_spans.append(spans)


def _preprocess(cfg, plan, x, rows, cols, vals, W1):
    """Build per-core input maps. All heavy lifting is layout/permutation."""
    L = plan.L
    # --- x shard, transposed + k-padded: [KPAD, NLOC] per core
    xT = np.zeros((cfg.CORES, cfg.KPAD, cfg.NLOC), dtype=np.float32)
    for c in range(cfg.CORES):
        xT[c, :cfg.NFEAT, :] = x[c * cfg.NLOC:(c + 1) * cfg.NLOC, :].T
    W1p = np.zeros((cfg.KPAD, cfg.NHID), dtype=np.float32)
    W1p[:cfg.NFEAT] = W1

    # --- edge data per (core, layer), ordered by call layout
    idx_arr = [[None] * L for _ in range(cfg.CORES)]
    row_arr = [[None] * L for _ in range(cfg.CORES)]
    val_arr = [[None] * L for _ in range(cfg.CORES)]
    for l in range(L):
        core_of = rows[l] // cfg.NLOC
        for c in range(cfg.CORES):
            m = core_of == c
            r = rows[l][m] - c * cfg.NLOC
            cc = cols[l][m]
            vv = vals[l][m]
            t_of = r // P
            s_of = cc // cfg.SUB
            order = np.argsort(t_of * cfg.NSUB + s_of, kind="stable")
            r, cc, vv = r[order], cc[order], vv[order]
            key = t_of[order] * cfg.NSUB + s_of[order]
            cnt = np.bincount(key, minlength=cfg.TILES * cfg.NSUB)
            off = np.zeros(cfg.TILES * cfg.NSUB + 1, dtype=np.int64)
            np.cumsum(cnt, out=off[1:])
            tot = plan.tot_ch[l] * P
            gidx = np.zeros(tot, dtype=np.int16)
            rloc = np.zeros(tot, dtype=np.float32)
            vpad = np.zeros(tot, dtype=np.float32)
            used = np.zeros(cfg.TILES * cfg.NSUB, dtype=np.int64)
            for gi, s, tl, ch_off in plan.calls[l]:
                o = ch_off * P
                for t, nch in tl:
                    k = t * cfg.NSUB + s
                    # (t,s) runs may span calls: continue from cursor
                    n = min(int(cnt[k]) - int(used[k]), nch * P)
                    if n > 0:
                        st = int(off[k]) + int(used[k])
                        sl = slice(st, st + n)
                        gidx[o:o + n] = (cc[sl] - s * cfg.SUB).astype(np.int16)
                        rloc[o:o + n] = (r[sl] - t * P).astype(np.float32)
                        vpad[o:o + n] = vv[sl]
                        used[k] += n
                    # pad slots keep gidx=0 (valid row of sub-table), val=0
                    o += nch * P
            # wrap indices: slot i -> [i%16, i//16], replicated to 128 parts
            w16 = gidx.reshape(-1, 16).T
            idx_arr[c][l] = np.tile(w16, (8, 1)).copy()
            row_arr[c][l] = rloc.reshape(-1, P).T.copy()
            val_arr[c][l] = vpad.reshape(-1, P).T.copy()
    return xT, W1p, idx_arr, row_arr, val_arr


def _build_program(cfg, plan, apply_b, apply_g, apply_lnb, apply_bout):
    import concourse.bacc as bacc
    import concourse.tile as tile
    import concourse.mybir as mybir
    from concourse import library_config

    f32 = mybir.dt.float32
    i16 = mybir.dt.int16
    Alu = mybir.AluOpType
    Act = mybir.ActivationFunctionType
    L, NH, NC = plan.L, cfg.NHID, cfg.NCLASS

    nc = bacc.Bacc("TRN2", target_bir_lowering=False, debug=False,
                   enable_asserts=False, num_devices=cfg.CORES,
                   num_swdge_queues=cfg.NQUEUES)

    # ---- I/O tensors
    xT_d = nc.dram_tensor("xT", [cfg.KPAD, cfg.NLOC], f32, kind="ExternalInput")
    W1_d = nc.dram_tensor("W1p", [cfg.KPAD, NH], f32, kind="ExternalInput")
    W2_d = nc.dram_tensor("W2", [NH, NH], f32, kind="ExternalInput")
    W3_d = nc.dram_tensor("W3", [NH, NH], f32, kind="ExternalInput")
    Wo_d = nc.dram_tensor("Wout", [NH, NC], f32, kind="ExternalInput")
    b_d = [nc.dram_tensor(f"b{l+1}", [P, NH], f32, kind="ExternalInput")
           for l in range(L)]
    bo_d = nc.dram_tensor("bout", [P, NC], f32, kind="ExternalInput")
    g_d = nc.dram_tensor("lng", [P, NH], f32, kind="ExternalInput")
    lb_d = nc.dram_tensor("lnb", [P, NH], f32, kind="ExternalInput")
    idx_d = [nc.dram_tensor(f"idx{l}", [P, plan.tot_ch[l] * 8], i16,
                            kind="ExternalInput") for l in range(L)]
    row_d = [nc.dram_tensor(f"rows{l}", [P, plan.tot_ch[l]], f32,
                            kind="ExternalInput") for l in range(L)]
    val_d = [nc.dram_tensor(f"vals{l}", [P, plan.tot_ch[l]], f32,
                            kind="ExternalInput") for l in range(L)]
    cst_d = nc.dram_tensor("consts", [P, 2 * P], f32, kind="ExternalInput")
    out_d = nc.dram_tensor("out", [cfg.NLOC, NC], f32, kind="ExternalOutput")

    W_next = [W2_d, W3_d]  # weight applied in tail of layers 0,1

    with tile.TileContext(nc) as tc:
        with tc.tile_pool(name="sb", bufs=1) as sb, \
             tc.tile_pool(name="sbG", bufs=3) as sbG, \
             tc.tile_pool(name="sbM", bufs=2) as sbM, \
             tc.tile_pool(name="sbX", bufs=2) as sbX, \
             tc.tile_pool(name="sbT", bufs=3) as sbT, \
             tc.tile_pool(name="psg", bufs=6, space="PSUM") as psg, \
             tc.tile_pool(name="pst", bufs=1, space="PSUM") as pst, \
             tc.tile_pool(name="dram", bufs=1, space="DRAM") as dram:

            nc.gpsimd.load_library(library_config.mlp)

            # ---- constants (iota row + identity, host-provided)
            cst_t = sb.tile([P, 2 * P], f32)
            nc.sync.dma_start(cst_t[:], cst_d[:])
            iota_t = cst_t[:, 0:P]
            ident = cst_t[:, P:2 * P]
            eps_t = sb.tile([P, 1], f32)
            nc.vector.memset(eps_t[:], cfg.EPS)
            W1_t = sb.tile([P, cfg.KCH * NH], f32)
            nc.sync.dma_start(
                W1_t[:].rearrange("p (k f) -> p k f", k=cfg.KCH),
                W1_d[:].rearrange("(k p) f -> p k f", p=P))
            Wn_t = []
            for l in range(L - 1):
                w = sb.tile([P, NH], f32, tag=f"wn{l}", name=f"wn{l}")
                nc.sync.dma_start(w[:], W_next[l][:])
                Wn_t.append(w)
            Wo_t = sb.tile([P, NC], f32)
            nc.sync.dma_start(Wo_t[:], Wo_d[:])
            b_t = []
            for l in range(L):
                if apply_b[l]:
                    t_ = sb.tile([P, NH], f32, tag=f"b{l}", name=f"bt{l}")
                    nc.sync.dma_start(t_[:], b_d[l][:])
                    b_t.append(t_)
                else:
                    b_t.append(None)
            g_t = lb_t = bo_t = None
            if apply_g:
                g_t = sb.tile([P, NH], f32)
                nc.sync.dma_start(g_t[:], g_d[:])
            if apply_lnb:
                lb_t = sb.tile([P, NH], f32)
                nc.sync.dma_start(lb_t[:], lb_d[:])
            if apply_bout:
                bo_t = sb.tile([P, NC], f32)
                nc.sync.dma_start(bo_t[:], bo_d[:])

            # ---- DRAM intermediates
            ag_in = [dram.tile([cfg.NLOC, NH], f32, tag=f"agin{l}",
                              name=f"agin{l}") for l in range(L)]
            aspace = "Shared" if cfg.CORES > 4 else "Local"
            pre_full = [dram.tile([cfg.N, NH], f32, tag=f"pref{l}",
                                  addr_space=aspace, name=f"pref{l}")
                        for l in range(L)]
            rg = [list(range(cfg.CORES))]

            # ---- phase 0: pre1 = x @ W1 (row-windowed)
            for w0 in range(0, cfg.NLOC, cfg.ROW_WINDOW):
                wl = min(cfg.ROW_WINDOW, cfg.NLOC - w0)
                xsl = sbX.tile([P, cfg.KCH * cfg.ROW_WINDOW], f32, tag="xsl")
                x3 = xsl[:].rearrange("p (k r) -> p k r", k=cfg.KCH)
                nc.sync.dma_start(
                    x3[:, :, :wl],
                    xT_d[:].rearrange("(k p) r -> p k r", p=P)[:, :, w0:w0 + wl])
                for t0 in range(0, wl, P):
                    t = (w0 + t0) // P
                    tl_ = cfg.tile_rows[t]
                    ps = psg.tile([P, NH], f32, space="PSUM", tag="ps")
                    for k in range(cfg.KCH):
                        nc.tensor.matmul(
                            ps[:tl_], lhsT=x3[:, k, t0:t0 + tl_],
                            rhs=W1_t[:, k * NH:(k + 1) * NH],
                            start=(k == 0), stop=(k == cfg.KCH - 1))
                    pre_sb = sbT.tile([P, NH], f32, tag="pre")
                    nc.scalar.copy(pre_sb[:tl_], ps[:tl_])
                    nc.sync.dma_start(ag_in[0][t * P:t * P + tl_, :],
                                      pre_sb[:tl_])

            # ---- layers
            for l in range(L):
                nc.gpsimd.collective_compute(
                    "AllGather", mybir.AluOpType.bypass,
                    ins=[ag_in[l].opt()], outs=[pre_full[l].opt()],
                    replica_groups=rg)

                spans = plan.group_spans[l]
                # per-tile chunk bookkeeping for start/stop flags
                tile_total = {t: int(plan.nch[l, t].sum())
                              for t in range(cfg.TILES)}
                tile_seen = {t: 0 for t in range(cfg.TILES)}
                psum_of = {}
                for gi, g in enumerate(cfg.groups):
                    lo, hi, cis = spans.get(gi, (0, 0, []))
                    gch = hi - lo
                    if gch > 0:
                        rows_t = sbM.tile([P, gch], f32, tag="rows")
                        nc.sync.dma_start(rows_t[:], row_d[l][:, lo:hi])
                        vals_t = sbM.tile([P, gch], f32, tag="vals")
                        nc.sync.dma_start(vals_t[:], val_d[l][:, lo:hi])
                        idx_t = sbM.tile([P, gch * 8], i16, tag="idx")
                        nc.sync.dma_start(idx_t[:], idx_d[l][:, lo * 8:hi * 8])

                    for ci in cis:
                        _, s, tl_list, ch_off = plan.calls[l][ci]
                        ncall = sum(c for _, c in tl_list)
                        nidx = ncall * P
                        G = sbG.tile([P, plan.CAP_CH * NH], f32, tag="G")
                        sub_rows = min(cfg.SUB, cfg.N - s * cfg.SUB)
                        nc.gpsimd.dma_gather(
                            G[:].rearrange("p (c f) -> p c f", f=NH)[:, :ncall, :],
                            pre_full[l][s * cfg.SUB:s * cfg.SUB + sub_rows, :],
                            idx_t[:, (ch_off - lo) * 8:(ch_off - lo + ncall) * 8],
                            nidx, nidx, NH,
                            queue_num=ci % cfg.NQUEUES)
                        cpos = 0
                        for t, nch in tl_list:
                            if t not in psum_of:
                                psum_of[t] = psg.tile([P, NH], f32, space="PSUM",
                                                      tag="ps", name=f"ps{t}")
                            pt = psum_of[t]
                            for j in range(nch):
                                ch = ch_off - lo + cpos + j
                                S_t = sbT.tile([P, P], f32, tag="S")
                                nc.vector.tensor_scalar(
                                    out=S_t[:], in0=iota_t,
                                    scalar1=rows_t[:, ch:ch + 1],
                                    scalar2=vals_t[:, ch:ch + 1],
                                    op0=Alu.is_equal, op1=Alu.mult)
                                nc.tensor.matmul(
                                    pt[:],
                                    lhsT=S_t[:],
                                    rhs=G[:, (cpos + j) * NH:(cpos + j + 1) * NH],
                                    start=(tile_seen[t] == 0),
                                    stop=(tile_seen[t] == tile_total[t] - 1),
                                    skip_group_check=True)
                                tile_seen[t] += 1
                            cpos += nch

                    # ---- tails for this group's tiles
                    for t in g:
                        tl_ = cfg.tile_rows[t]
                        if tile_total[t] == 0:
                            m_in = sbT.tile([P, NH], f32, tag="h")
                            nc.vector.memset(m_in[:], 0.0)
                        else:
                            m_in = psum_of.pop(t)
                        # h = m (+ b)
                        if b_t[l] is not None:
                            h_t = sbT.tile([P, NH], f32, tag="h2")
                            nc.vector.tensor_tensor(out=h_t[:], in0=m_in[:],
                                                    in1=b_t[l][:], op=Alu.add)
                        else:
                            h_t = m_in
                        stats6 = sbT.tile([P, 6], f32, tag="st6")
                        nc.vector.bn_stats(stats6[:], h_t[:])
                        stats2 = sbT.tile([P, 2], f32, tag="st2")
                        nc.vector.bn_aggr(stats2[:], stats6[:])
                        std_t = sbT.tile([P, 1], f32, tag="std")
                        nc.scalar.activation(std_t[:], stats2[:, 1:2],
                                             Act.Sqrt, bias=eps_t[:, 0:1])
                        inv_t = sbT.tile([P, 1], f32, tag="inv")
                        nc.vector.reciprocal(inv_t[:], std_t[:])
                        nms = sbT.tile([P, 1], f32, tag="nms")
                        nc.vector.tensor_scalar(
                            out=nms[:], in0=stats2[:, 0:1],
                            scalar1=inv_t[:, 0:1], scalar2=-1.0,
                            op0=Alu.mult, op1=Alu.mult)
                        t_n = sbT.tile([P, NH], f32, tag="tn")
                        nc.vector.tensor_scalar(
                            out=t_n[:], in0=h_t[:], scalar1=inv_t[:, 0:1],
                            scalar2=nms[:, 0:1], op0=Alu.mult, op1=Alu.add)
                        if g_t is not None:
                            t_g = sbT.tile([P, NH], f32, tag="tg")
                            nc.vector.tensor_tensor(out=t_g[:], in0=t_n[:],
                                                    in1=g_t[:], op=Alu.mult)
                            t_n = t_g
                        if lb_t is not None:
                            t_b = sbT.tile([P, NH], f32, tag="tb")
                            nc.vector.tensor_tensor(out=t_b[:], in0=t_n[:],
                                                    in1=lb_t[:], op=Alu.add)
                            t_n = t_b
                        hr = sbT.tile([P, NH], f32, tag="hr")
                        nc.scalar.activation(hr[:], t_n[:], Act.Relu)
                        # transpose
                        hT_ps = pst.tile([P, P], f32, space="PSUM", tag="ptr")
                        nc.tensor.transpose(out=hT_ps[:], in_=hr[:],
                                            identity=ident)
                        hT = sbT.tile([P, P], f32, tag="hT")
                        nc.scalar.copy(hT[:], hT_ps[:])
                        if l < L - 1:
                            pr_ps = pst.tile([P, NH], f32, space="PSUM",
                                             tag="ppr")
                            nc.tensor.matmul(pr_ps[:tl_], lhsT=hT[:, :tl_],
                                             rhs=Wn_t[l][:],
                                             start=True, stop=True)
                            pre_sb = sbT.tile([P, NH], f32, tag="pre")
                            nc.scalar.copy(pre_sb[:tl_], pr_ps[:tl_])
                            nc.sync.dma_start(
                                ag_in[l + 1][t * P:t * P + tl_, :],
                                pre_sb[:tl_])
                        else:
                            lg_ps = pst.tile([P, NC], f32, space="PSUM",
                                             tag="ppr")
                            nc.tensor.matmul(lg_ps[:tl_], lhsT=hT[:, :tl_],
                                             rhs=Wo_t[:], start=True,
                                             stop=True)
                            lg = sbT.tile([P, NC], f32, tag="lg")
                            if bo_t is not None:
                                nc.vector.tensor_tensor(
                                    out=lg[:tl_], in0=lg_ps[:tl_],
                                    in1=bo_t[:tl_], op=Alu.add)
                            else:
                                nc.vector.tensor_copy(lg[:tl_], lg_ps[:tl_])
                            mx = sbT.tile([P, 1], f32, tag="mx")
                            nc.vector.reduce_max(mx[:tl_], lg[:tl_],
                                                 axis=mybir.AxisListType.X)
                            nmx = sbT.tile([P, 1], f32, tag="nmx")
                            nc.vector.tensor_scalar(
                                out=nmx[:tl_], in0=mx[:tl_], scalar1=-1.0,
                                scalar2=None, op0=Alu.mult)
                            ex = sbT.tile([P, NC], f32, tag="ex")
                            se = sbT.tile([P, 1], f32, tag="se")
                            nc.scalar.activation(ex[:tl_], lg[:tl_], Act.Exp,
                                                 bias=nmx[:tl_, 0:1],
                                                 scale=1.0,
                                                 accum_out=se[:tl_, 0:1])
                            lse = sbT.tile([P, 1], f32, tag="lse")
                            nc.scalar.activation(lse[:tl_], se[:tl_], Act.Ln)
                            sh = sbT.tile([P, 1], f32, tag="sh")
                            nc.vector.tensor_tensor(out=sh[:tl_],
                                                    in0=mx[:tl_],
                                                    in1=lse[:tl_],
                                                    op=Alu.add)
                            res = sbT.tile([P, NC], f32, tag="res")
                            nc.vector.tensor_scalar(
                                out=res[:tl_], in0=lg[:tl_],
                                scalar1=sh[:tl_, 0:1], scalar2=None,
                                op0=Alu.subtract)
                            nc.sync.dma_start(
                                out_d[t * P:t * P + tl_, :], res[:tl_])
    return nc


def _prepare(x, rows, cols, vals, W1, b1, W2, b2, W3, b3,
             ln_g, ln_b, Wout, bout, cfg):
    x = np.ascontiguousarray(np.asarray(x, dtype=np.float32))
    rows = np.asarray(rows, dtype=np.int64)
    cols = np.asarray(cols, dtype=np.int64)
    vals = np.asarray(vals, dtype=np.float32)
    W1 = np.asarray(W1, dtype=np.float32)
    plan = _Plan(cfg, rows, cols)
    xT, W1p, idx_arr, row_arr, val_arr = _preprocess(
        cfg, plan, x, rows, cols, vals, W1)

    rep = np.ones((P, 1), np.float32)
    b_np = [np.asarray(b, np.float32) for b in (b1, b2, b3)]
    apply_b = [not np.all(b == 0) for b in b_np]
    ln_g = np.asarray(ln_g, np.float32)
    ln_b = np.asarray(ln_b, np.float32)
    bout = np.asarray(bout, np.float32)
    apply_g = not np.all(ln_g == 1)
    apply_lnb = not np.all(ln_b == 0)
    apply_bout = not np.all(bout == 0)

    nc = _build_program(cfg, plan, apply_b, apply_g, apply_lnb, apply_bout)
    nc.compile()

    in_maps = []
    for c in range(cfg.CORES):
        consts = np.concatenate(
            [np.tile(np.arange(P, dtype=np.float32)[None, :], (P, 1)),
             np.eye(P, dtype=np.float32)], axis=1)
        m = {
            "xT": xT[c],
            "W1p": W1p,
            "consts": np.ascontiguousarray(consts),
            "W2": np.asarray(W2, np.float32),
            "W3": np.asarray(W3, np.float32),
            "Wout": np.asarray(Wout, np.float32),
            "bout": np.ascontiguousarray(rep * bout[None, :]),
            "lng": np.ascontiguousarray(rep * ln_g[None, :]),
            "lnb": np.ascontiguousarray(rep * ln_b[None, :]),
        }
        for l in range(3):
            m[f"b{l+1}"] = np.ascontiguousarray(rep * b_np[l][None, :])
            m[f"idx{l}"] = idx_arr[c][l]
            m[f"rows{l}"] = row_arr[c][l]
            m[f"vals{l}"] = val_arr[c][l]
        in_maps.append(m)

    return nc, in_maps


def kernel(**inputs):
    from concourse.bass_utils import run_bass_kernel_spmd
    cfg = _Cfg(N=100000, NFEAT=602, NHID=128, NCLASS=41, CORES=8)
    nc, in_maps = _prepare(cfg=cfg, **inputs)
    res = run_bass_kernel_spmd(nc, in_maps, core_ids=list(range(cfg.CORES)))
    out = np.concatenate([r["out"] for r in res.results], axis=0)
    return np.ascontiguousarray(out)
